# revision 51
# baseline (speedup 1.0000x reference)
"""Trainium2 Bass kernel for ViT-style attention block with RoPE.

Problem: x(64,197,1024), qkv(3072x1024)+b, proj(1024x1024)+b, H=16 heads,
RoPE (interleaved pairs, tiled cos/sin tables) on all tokens but CLS.

Strategy: data-parallel over batch across 8 cores (8 items each, no
collectives). Host pre-transposes all operands so the device only runs
matmuls / softmax / RoPE in "transposed" layouts:

  - qk part:  qkT[f, t] = Wqk^T stationary x xT moving   (features on partitions)
  - v part:   v[t, f]   = xT stationary x Wv moving      (tokens on partitions)
  - scores:   scT[j, i] = kT(lhsT) x qT(rhs), two heads packed into PE rows
              0:64 / 64:128, both jt-tiles packed into one PSUM bank per head
  - softmax:  exp on ScalarE (scale=1/8, no max subtraction; |logits|<~5),
              denominators via ones-columns appended to v (rows 64:128 of AV
              psum), normalization = reciprocal straight off PSUM + DVE mult
  - RoPE:     q' = (q+b)*cos + (P(q+b))*sinS where P is a 128x128 block-swap
              permutation done on the TensorEngine; sign and d-permutation
              folded into host-built tables
  - v bias:   folded into proj bias on host (attn rows sum to 1)
  - proj:     yT = Wproj^T stationary x concatT moving (394-col chains),
              bias on ScalarE

The emission is a single software pipeline: 64 "waves" (scores->exp->AV->
normalize, one head-pair x item each) are interleaved with independent
"chain" tasks (QKV f-tile chains, V chains, proj chains) at ~2 chains per
wave so the TensorEngine never idles waiting on ScalarE/DVE results. DMAs
are split across the two HW DGE queues (sync + scalar) with priority
ordering so the first matmul can start ~4us in.
"""

import sys

for _p in ("/opt/trn_rl_repo", "/opt/pypackages"):
    if _p not in sys.path:
        sys.path.append(_p)

import numpy as np
import ml_dtypes

import concourse.bass as bass
import concourse.tile as tile
from concourse import bacc
from concourse import mybir

F32 = mybir.dt.float32
BF16 = mybir.dt.bfloat16
BF16_NP = ml_dtypes.bfloat16

# Problem constants (hardcoded per the contract)
B, N, C = 64, 197, 1024
H, D = 16, 64
E = 1  # CLS tokens
THETA = 10000.0
N_CORES = 8
NI = B // N_CORES  # items per core = 8
NT = NI * N  # tokens per core = 1576
S = N  # 197
W = 2 * S  # pair width = 394
NPAIR = NI // 2  # 4
P = 128

# f-tile emission order: q,k interleaved so waves unlock early
FO = [0, 8, 1, 9, 2, 10, 3, 11, 4, 12, 5, 13, 6, 14, 7, 15]


def _host_tables():
    """RoPE cos/sin in device layout + permutations, all position-only."""
    seq = (224 // 16) ** 2  # 196
    exp = np.arange(0, D, 2, dtype=np.float64) / -D
    base = THETA**exp  # (32,)
    t = np.arange(seq, dtype=np.float64)
    f0 = np.outer(t, base)  # (196, 32)
    f = np.concatenate([f0, f0], axis=-1)  # (196, 64) "tiled"
    cos_ref = np.cos(f)
    sin_ref = np.sin(f)

    # permutation: new dd<32 -> orig 2dd (x0), new dd>=32 -> orig 2(dd-32)+1 (x1)
    perm = np.empty(D, dtype=np.int64)
    perm[:32] = np.arange(32) * 2
    perm[32:] = np.arange(32) * 2 + 1

    # per-token columns for an item: col 0 = CLS (cos=1, sin=0), cols 1..196 = rope
    cos_item = np.ones((D, S), dtype=np.float64)
    sin_item = np.zeros((D, S), dtype=np.float64)
    cos_item[:, 1:] = cos_ref[:, perm].T
    sin_item[:, 1:] = sin_ref[:, perm].T
    # fold rotate-half signs into sin: rot[dd<32] = -q[dd+32], rot[dd>=32] = +q[dd-32]
    sinS_item = sin_item.copy()
    sinS_item[:32, :] *= -1.0

    # pair-width, replicated for the 2 heads in a 128-partition tile
    cosT = np.tile(cos_item, (2, 2)).astype(BF16_NP)  # [128, 394]
    sinST = np.tile(sinS_item, (2, 2)).astype(BF16_NP)  # [128, 394]

    # 128x128 swap permutation (block swap +-32 within each 64-head-half),
    # already transposed for use as lhsT: rot = P @ q  ->  lhsT = P.T
    Pm = np.zeros((P, P), dtype=np.float32)
    for p in range(P):
        src = 64 * (p // 64) + ((p % 64) + 32) % 64
        Pm[p, src] = 1.0
    pmatT = Pm.T.astype(BF16_NP)  # [K=128, M=128]

    return perm, cosT, sinST, pmatT


def _pack_weights(qkv_w, qkv_b, proj_w, proj_b, perm):
    """Host-side weight packing into device layouts (all numpy, one-time)."""
    # feature permutation for q/k heads: rows of qkv_w within each head
    qk_perm = np.concatenate(
        [h * D + perm for h in range(2 * H)]  # q heads then k heads
    )
    wqk = qkv_w[:2048][qk_perm]  # (2048, 1024) permuted
    bqk = qkv_b[:2048][qk_perm]  # (2048,)
    wv = qkv_w[2048:]  # (1024, 1024)
    bv = qkv_b[2048:]

    wqk_T = np.ascontiguousarray(wqk.T).astype(BF16_NP)  # [1024, 2048]
    # chunk-major repack: chunk ci = f-tile FO[ci], laid out [p, o, f] and
    # flattened to [128, 16*1024] so each chunk DMA reads contiguous 2KB
    # per-partition lines (the startup-critical stream)
    blocks = []
    for ft in FO:
        blk = wqk_T.reshape(8, P, 2048)[:, :, ft * P : (ft + 1) * P]  # [o, p, f]
        blocks.append(np.transpose(blk, (1, 0, 2)).reshape(P, 8 * P))
    wqkc = np.ascontiguousarray(np.concatenate(blocks, axis=1))  # [128, 16384]
    wv_T = np.ascontiguousarray(wv.T).astype(BF16_NP)  # [1024, 1024]
    # half-major repack for contiguous per-partition DMA lines
    wvh = np.concatenate(
        [
            np.transpose(
                wv_T.reshape(8, P, 1024)[:, :, nk * 512 : (nk + 1) * 512], (1, 0, 2)
            ).reshape(P, 8 * 512)
            for nk in range(2)
        ],
        axis=1,
    )  # [128, 8192]
    wvh = np.ascontiguousarray(wvh)
    proj_wT = np.ascontiguousarray(proj_w.T).astype(BF16_NP)  # [1024, 1024]
    wprh = np.ascontiguousarray(
        np.transpose(proj_wT.reshape(8, P, 1024), (1, 0, 2)).reshape(P, 8 * 1024)
    )  # [128, 8192] contiguous [p, o, f] lines

    # biases in [128, ftile] per-partition layout
    bqk_dev = np.ascontiguousarray(bqk.reshape(16, 128).T).astype(np.float32)
    # v bias folded into proj bias: y = concat@W^T + (W@bv + pb)
    beff = proj_w.astype(np.float64) @ bv.astype(np.float64) + proj_b
    beff_dev = np.ascontiguousarray(beff.reshape(8, 128).T).astype(np.float32)
    return wqkc, wvh, wprh, bqk_dev, beff_dev


def build_nc(n_items=NI):
    """Build the per-core Bass graph. SPMD: same graph on all cores."""
    assert n_items == NI
    npair = n_items // 2
    nt = n_items * S
    nc = bacc.Bacc(None, target_bir_lowering=False, debug=False)

    xT = nc.declare_dram_parameter("xT", [C, nt], BF16, isOutput=False)
    wqk = nc.declare_dram_parameter("wqk", [P, 16 * 8 * P], BF16, isOutput=False)
    wv = nc.declare_dram_parameter("wv", [P, 2 * 8 * 512], BF16, isOutput=False)
    wpr = nc.declare_dram_parameter("wpr", [P, 8 * 1024], BF16, isOutput=False)
    pmat = nc.declare_dram_parameter("pmat", [P, P], BF16, isOutput=False)
    bqk = nc.declare_dram_parameter("bqk", [P, 16], F32, isOutput=False)
    beff = nc.declare_dram_parameter("beff", [P, 8], F32, isOutput=False)
    cosT = nc.declare_dram_parameter("cosT", [P, W], BF16, isOutput=False)
    sinST = nc.declare_dram_parameter("sinST", [P, W], BF16, isOutput=False)
    out = nc.declare_dram_parameter("out", [C, nt], F32, isOutput=True)

    Exp = mybir.ActivationFunctionType.Exp
    Ident = mybir.ActivationFunctionType.Identity

    with tile.TileContext(nc) as tc:
        with (
            tc.tile_pool(name="const", bufs=1) as const,
            tc.tile_pool(name="xp", bufs=2) as xp,
            tc.tile_pool(name="roped", bufs=2) as rp,
            tc.tile_pool(name="work", bufs=4) as wk,
            tc.tile_pool(name="ep", bufs=3) as ep,
            tc.tile_pool(name="rbp", bufs=2) as rbp,
            tc.tile_pool(name="cc", bufs=3) as cc,
            tc.tile_pool(name="yp", bufs=3) as yp,
            tc.tile_pool(name="psS", bufs=1, space="PSUM") as psS,
            tc.tile_pool(name="psV", bufs=3, space="PSUM") as psV,
            tc.tile_pool(name="psC", bufs=3, space="PSUM") as psC,
        ):
            # ---- persistent tiles ----
            xT3 = xT.rearrange("(o p) t -> p o t", p=P)
            out3 = out.rearrange("(o p) t -> p o t", p=P)
            wqkc4 = wqk.rearrange("p (c o f) -> p c o f", c=16, f=P)

            wqk_sb = const.tile([P, 8, 2048], BF16)
            wv_sb = const.tile([P, 8, C], BF16)
            wpr_sb = const.tile([P, 8, C], BF16)
            pmat_sb = const.tile([P, P], BF16)
            bqk_sb = const.tile([P, 16], F32)
            beff_sb = const.tile([P, 8], F32)
            cos_sb = const.tile([P, W], BF16)
            sin_sb = const.tile([P, W], BF16)
            # v65[set][it2][tt]: persistent, ones in cols 64:128 set once
            v65 = [
                [
                    [
                        const.tile([P, 16, P], BF16, name=f"v65_{st}_{i2}_{tt}")
                        for tt in range(2)
                    ]
                    for i2 in range(2)
                ]
                for st in range(2)
            ]

            # ---- DMA priority order ----
            # sync queue carries the startup-critical bytes in exact
            # need-order; small consts go on the scalar HWDGE queue; the
            # proj weights (needed ~100us in) are DMA'd later (see
            # wpr_dma chain task) so they don't steal startup bandwidth.
            wvh4 = wv.rearrange("p (n o f) -> p n o f", n=2, f=512)
            x_tiles = [None] * npair

            def _wqk_chunk(ci):
                ft = FO[ci]
                nc.sync.dma_start(
                    wqk_sb[:, :, ft * P : (ft + 1) * P], wqkc4[:, ci]
                )

            # sync queue: ft0 chunk FIRST so the first chain streams k-tile by
            # k-tile as x lands (in-order queue: putting it after x0 would make
            # the first LDWEIGHTS wait for all of x0). x0 split across both
            # HWDGE queues — each tops out ~190GB/s, together ~360.
            _wqk_chunk(0)
            x_tiles[0] = xp.tile([P, 8, W], BF16, tag="x", name="x_sb")
            for kt in range(0, 8, 2):
                nc.sync.dma_start(x_tiles[0][:, kt], xT3[:, kt, 0:W])
            for kt in range(1, 8, 2):
                nc.scalar.dma_start(x_tiles[0][:, kt], xT3[:, kt, 0:W])
            for ci in range(1, 16):
                _wqk_chunk(ci)
            x_tiles[1] = xp.tile([P, 8, W], BF16, tag="x", name="x_sb")
            nc.sync.dma_start(x_tiles[1], xT3[:, :, W : 2 * W])
            # scalar queue (runs concurrently with sync queue): small consts,
            # then wv in halves so the early V chains unblock sooner
            nc.scalar.dma_start(bqk_sb, bqk[:, :])
            nc.scalar.dma_start(pmat_sb, pmat[:, :])
            nc.scalar.dma_start(cos_sb, cosT[:, :])
            nc.scalar.dma_start(sin_sb, sinST[:, :])
            nc.scalar.dma_start(beff_sb, beff[:, :])
            nc.scalar.dma_start(wv_sb[:, :, 0:512], wvh4[:, 0])
            nc.scalar.dma_start(wv_sb[:, :, 512:1024], wvh4[:, 1])
            # ones columns of v65, once
            for st in range(2):
                for i2 in range(2):
                    for tt in range(2):
                        nc.gpsimd.memset(v65[st][i2][tt][:, :, 64:128], 1.0)

            roped_tiles = [None] * npair
            concat_tiles = [None] * npair
            tmp_tiles = {}
            e_tiles = {}

            # ---- task emitters ----
            def emit_qk_chain(p, ft):
                ps = psC.tile([P, 512], F32, tag="c", name="ps_c")
                x_sb = x_tiles[p]
                for kt in range(8):
                    nc.tensor.matmul(
                        ps[:, :W],
                        wqk_sb[:, kt, ft * P : (ft + 1) * P],
                        x_sb[:, kt, :],
                        start=(kt == 0),
                        stop=(kt == 7),
                    )
                t = wk.tile([P, W], BF16, tag="tmp", name="tmp")
                nc.scalar.activation(t, ps[:, :W], Ident, bias=bqk_sb[:, ft : ft + 1])
                tmp_tiles[(p, ft)] = t

            def emit_rope(p, ft):
                t = tmp_tiles.pop((p, ft))
                ps_rot = psC.tile([P, 512], F32, tag="c", name="ps_c")
                nc.tensor.matmul(ps_rot[:, :W], pmat_sb, t, start=True, stop=True)
                acc = wk.tile([P, W], BF16, tag="acc", name="acc")
                nc.vector.tensor_mul(acc, t, cos_sb)
                rot2 = wk.tile([P, W], BF16, tag="rot2", name="rot2")
                nc.vector.tensor_mul(rot2, ps_rot[:, :W], sin_sb)
                nc.vector.tensor_add(roped_tiles[p][:, ft, :], acc, rot2)

            def emit_v_half(p, it2, tt, nk):
                pcount = P if tt == 0 else S - P
                ps = psC.tile([P, 512], F32, tag="c", name="ps_c")
                x_sb = x_tiles[p]
                base = it2 * S + tt * P
                for kt in range(8):
                    nc.tensor.matmul(
                        ps[:pcount, :],
                        x_sb[:, kt, base : base + pcount],
                        wv_sb[:, kt, nk * 512 : (nk + 1) * 512],
                        start=(kt == 0),
                        stop=(kt == 7),
                    )
                vt = v65[p % 2][it2][tt]
                nc.vector.tensor_copy(
                    vt[:pcount, nk * 8 : (nk + 1) * 8, 0:64],
                    ps[:pcount, :].rearrange("p (h d) -> p h d", d=64),
                )

            def emit_scores(p, hp, it2):
                ts = it2 * S
                rt = roped_tiles[p]
                sc = psS.tile([P, 2, 512], F32, tag="sc", name="sc")
                for bk in range(2):
                    hb = 64 * bk  # head 2hp -> rows 0:64, 2hp+1 -> 64:128
                    kT = rt[hb : hb + 64, 8 + hp, ts : ts + S]
                    qT = rt[hb : hb + 64, hp, ts : ts + S]
                    nc.tensor.matmul(
                        sc[:, bk, 0:S], kT[:, 0:P], qT, start=True, stop=True
                    )
                    nc.tensor.matmul(
                        sc[0:69, bk, 256 : 256 + S],
                        kT[:, P:S],
                        qT,
                        start=True,
                        stop=True,
                    )
                e = ep.tile([P, 2, 2, S], BF16, tag="e", name="e")
                nc.scalar.activation(e[:, :, 0, :], sc[:, :, 0:S], Exp, scale=0.125)
                nc.scalar.activation(
                    e[0:69, :, 1, :], sc[0:69, :, 256 : 256 + S], Exp, scale=0.125
                )
                e_tiles[(p, hp, it2)] = e

            def emit_av(p, hp, it2):
                ts = it2 * S
                e = e_tiles.pop((p, hp, it2))
                vset = v65[p % 2][it2]
                av = psV.tile([P, 2, 256], F32, tag="av", name="av")
                for bk in range(2):
                    h = 2 * hp + bk
                    nc.tensor.matmul(
                        av[:, bk, 0:S],
                        vset[0][:, h, :],
                        e[:, bk, 0, :],
                        start=True,
                        stop=False,
                    )
                    nc.tensor.matmul(
                        av[:, bk, 0:S],
                        vset[1][0:69, h, :],
                        e[0:69, bk, 1, :],
                        start=False,
                        stop=True,
                    )
                ssum = rbp.tile([64, 2, S], F32, tag="ssum", name="ssum")
                nc.scalar.copy(ssum, av[64:128, :, 0:S])
                rb = rbp.tile([64, 2, S], F32, tag="rb", name="rb")
                nc.vector.reciprocal_approx_fast(rb, ssum)
                for bk in range(2):
                    nc.vector.tensor_mul(
                        concat_tiles[p][64 * bk : 64 * bk + 64, hp, ts : ts + S],
                        av[0:64, bk, 0:S],
                        rb[:, bk, :],
                    )

            def emit_proj(p, ft):
                ps = psC.tile([P, 512], F32, tag="c", name="ps_c")
                for kt in range(8):
                    nc.tensor.matmul(
                        ps[:, 0:W],
                        wpr_sb[:, kt, ft * P : (ft + 1) * P],
                        concat_tiles[p][:, kt, 0:W],
                        start=(kt == 0),
                        stop=(kt == 7),
                    )
                y = yp.tile([P, W], F32, tag="y", name="y")
                nc.scalar.activation(y, ps[:, 0:W], Ident, bias=beff_sb[:, ft : ft + 1])
                nc.sync.dma_start(out3[:, ft, p * W : (p + 1) * W], y)

            def emit_proj2(p, it2, ft):
                # per-item 197-col proj chain: lets the last pair's it2=0 proj
                # run as wave filler instead of a serial tail
                ts = it2 * S
                ps = psC.tile([P, 512], F32, tag="c", name="ps_c")
                for kt in range(8):
                    nc.tensor.matmul(
                        ps[:, 0:S],
                        wpr_sb[:, kt, ft * P : (ft + 1) * P],
                        concat_tiles[p][:, kt, ts : ts + S],
                        start=(kt == 0),
                        stop=(kt == 7),
                    )
                y = yp.tile([P, S], F32, tag="y2", name="y2")
                nc.scalar.activation(y, ps[:, 0:S], Ident, bias=beff_sb[:, ft : ft + 1])
                nc.sync.dma_start(out3[:, ft, p * W + ts : p * W + ts + S], y)

            # ---- chain-task schedule ----
            def alloc_pair(p):
                roped_tiles[p] = rp.tile([P, 16, W], BF16, tag="roped", name="roped")
                concat_tiles[p] = cc.tile([P, 8, W], BF16, tag="cc", name="cc")

            def pair_tasks(p):
                """25 chain tasks for pair p (qk chains w/ trailing rope, v halves)."""
                qs = [("qk", p, i) for i in range(16)]
                vs = [("v", p, i2, tt, nk) for nk in range(2) for i2 in range(2)
                      for tt in range(2)]
                # interleave: qk0,qk1,v0,v1, qk2,qk3,v2,v3, qk4..5+v4..7, rest qk
                order = [
                    qs[0], qs[1], vs[0], vs[1],
                    qs[2], qs[3], vs[2], vs[3],
                    qs[4], qs[5], vs[4], vs[5], vs[6], vs[7],
                    qs[6], qs[7], qs[8], qs[9], qs[10], qs[11],
                    qs[12], qs[13], qs[14], qs[15],
                    ("rope_tail", p),
                ]
                return order

            def run_task(t):
                kind = t[0]
                if kind == "qk":
                    _, p, i = t
                    if i == 0:
                        alloc_pair(p)
                    emit_qk_chain(p, FO[i])
                    if i > 0:
                        emit_rope(p, FO[i - 1])
                elif kind == "v":
                    _, p, i2, tt, nk = t
                    emit_v_half(p, i2, tt, nk)
                elif kind == "rope_tail":
                    _, p = t
                    emit_rope(p, FO[15])
                elif kind == "proj":
                    _, p, ft = t
                    emit_proj(p, ft)
                elif kind == "proj2":
                    _, p, it2, ft = t
                    emit_proj2(p, it2, ft)
                elif kind == "xdma":
                    _, p = t
                    x_tiles[p] = xp.tile([P, 8, W], BF16, tag="x", name="x_sb")
                    nc.sync.dma_start(x_tiles[p], xT3[:, :, p * W : (p + 1) * W])
                elif kind == "wpr_dma":
                    nc.scalar.dma_start(wpr_sb, wpr.rearrange("p (o f) -> p o f", o=8))

            def interleave(a, b, period, offset):
                """Insert items of b into a, one every `period`, starting at offset."""
                res = list(a)
                pos = offset
                for item in b:
                    pos = min(pos, len(res))
                    res.insert(pos, item)
                    pos += period + 1
                return res

            chain_list = []
            chain_list += pair_tasks(0)
            chain_list += pair_tasks(1)
            blk = pair_tasks(2)
            blk = interleave(blk, [("xdma", 3)], 0, 2)
            chain_list += interleave(blk, [("proj", 0, ft) for ft in range(8)], 2, 3)
            blk = pair_tasks(3)
            chain_list += interleave(blk, [("proj", 1, ft) for ft in range(8)], 2, 3)
            chain_list += [("proj", 2, ft) for ft in range(8)]
            tail_tasks = [("proj", 3, ft) for ft in range(8)]
            pinned = {}
            # x(p2) DMA early in pair-1 block; wpr DMA once startup drains
            chain_list = interleave(chain_list, [("xdma", 2)], 0, 27)
            chain_list = interleave(chain_list, [("wpr_dma",)], 0, 16)

            waves = [
                (p, hp, it2)
                for p in range(npair)
                for hp in range(8)
                for it2 in range(2)
            ]

            # prologue: 8 chain tasks before the first wave
            ci = 0
            for _ in range(8):
                run_task(chain_list[ci])
                ci += 1

            # steady pipeline: scores(k) | filler | av(k-1) | filler
            n_w = len(waves)
            prev = None
            for k in range(n_w + 1):
                if k < n_w:
                    emit_scores(*waves[k])
                budget = 2 if k < n_w else 0
                if k in pinned:
                    run_task(pinned[k])
                    budget = 0
                if ci < len(chain_list) and budget:
                    run_task(chain_list[ci])
                    ci += 1
                    budget -= 1
                if prev is not None:
                    emit_av(*prev)
                prev = waves[k] if k < n_w else None
                while budget and ci < len(chain_list):
                    run_task(chain_list[ci])
                    ci += 1
                    budget -= 1
            while ci < len(chain_list):
                run_task(chain_list[ci])
                ci += 1
            for t in tail_tasks:
                run_task(t)

    nc.compile()
    return nc


def host_pack_inputs(x, qkv_w, qkv_b, proj_w, proj_b, n_items=NI):
    """Build per-core in_maps (host-side layout only, no math on x)."""
    perm, cosT, sinST, pmatT = _host_tables()
    wqk_T, wv_T, proj_wT, bqk_dev, beff_dev = _pack_weights(
        qkv_w, qkv_b, proj_w, proj_b, perm
    )
    shared = {
        "wqk": wqk_T,
        "wv": wv_T,
        "wpr": proj_wT,
        "pmat": np.ascontiguousarray(pmatT),
        "bqk": bqk_dev,
        "beff": beff_dev,
        "cosT": np.ascontiguousarray(cosT),
        "sinST": np.ascontiguousarray(sinST),
    }
    n_cores = x.shape[0] // n_items
    in_maps = []
    for c in range(n_cores):
        xs = x[c * n_items : (c + 1) * n_items]  # [ni, 197, 1024]
        xTs = np.ascontiguousarray(
            xs.reshape(n_items * S, C).T.astype(BF16_NP)
        )  # [1024, nt]
        in_maps.append({"xT": xTs, **shared})
    return in_maps


def unpack_output(results, n_items=NI):
    """results: list of per-core {'out': [1024, nt]} -> full (B, N, C) f32."""
    outs = []
    for r in results:
        yT = r["out"]  # [1024, nt]
        outs.append(yT.T.reshape(n_items, S, C))
    return np.concatenate(outs, axis=0)


_CACHED = {}


def kernel(x, qkv_w, qkv_b, proj_w, proj_b):
    from concourse.bass_utils import run_bass_kernel_spmd

    x = np.asarray(x, dtype=np.float32)
    qkv_w = np.asarray(qkv_w, dtype=np.float32)
    qkv_b = np.asarray(qkv_b, dtype=np.float32)
    proj_w = np.asarray(proj_w, dtype=np.float32)
    proj_b = np.asarray(proj_b, dtype=np.float32)

    if "nc" not in _CACHED:
        _CACHED["nc"] = build_nc(NI)
    nc = _CACHED["nc"]
    in_maps = host_pack_inputs(x, qkv_w, qkv_b, proj_w, proj_b, NI)
    res = run_bass_kernel_spmd(nc, in_maps, core_ids=list(range(N_CORES)))
    return unpack_output(res.results, NI).astype(np.float32)


if __name__ == "__main__":
    pass
